# revision 2
# baseline (speedup 1.0000x reference)
"""Trainium2 Bass kernel for MFVIConstituency mean-field iterations.

Per batch b (one NeuronCore each, 8 total):
    q = s_con;  repeat 3x:  q[i,j] = s_con[i,j] + sum_k sig(q)[j,k] * sb[i,j,k]
    out = sigmoid(q)
where sb = s_bin * mask2o, mask2o[i,j,k] = mask[i,j] & (i!=k) & (j!=k).

Host (numpy) does: masking, fp16 cast, SBUF-cache layout packing, iteration-1
sigmoid, final transpose. Device does: 3x [fp16 tensor_tensor mul (DVE 2x mode)
-> split DVE/ACT segmented reduce] + sigmoid + xbar-transpose DMAs to rebuild
the sig operand layout between iterations.

On-chip layout: q is assembled transposed (QT[j,i]); j lives on partitions in
two chunks: chunk1 = j 0:128, chunk2 "packed" = j 128:192 duplicated across
both partition halves with the i-range split (p<64: i 0:96, p>=64: i 96:192)
so every DVE instruction uses all 128 partitions.
"""

import numpy as np

S = 192
B = 8
P = 128
G = 48          # i-values per slab -> slab free size G*S = 9216
NSLAB1 = 4      # chunk1: 4 slabs of 48 i-values (j in 0:128)
NSLAB2 = 2      # chunk2 packed: 96 i-per-half * 2 halves / 48
DVE_SEGS = 30   # of the G=48 segments per slab, how many the DVE reduces
                # (rest go to the scalar/ACT engine via activation accum)

_CACHE = {}


def _build_program():
    import concourse.tile as tile
    from concourse import mybir, bacc
    from contextlib import ExitStack

    f32, f16 = mybir.dt.float32, mybir.dt.float16
    SLAB = G * S

    nc = bacc.Bacc("TRN2", target_bir_lowering=False, debug=False, num_devices=B)
    c1_d = nc.dram_tensor("c1", [P, NSLAB1 * SLAB], f16, kind="ExternalInput")
    c2_d = nc.dram_tensor("c2", [P, NSLAB2 * SLAB], f16, kind="ExternalInput")
    siga_d = nc.dram_tensor("siga", [P, S], f16, kind="ExternalInput")
    sigb_d = nc.dram_tensor("sigb", [P, S], f16, kind="ExternalInput")
    sconT1_d = nc.dram_tensor("sconT1", [P, S], f32, kind="ExternalInput")
    sconT2p_d = nc.dram_tensor("sconT2p", [P, 96], f32, kind="ExternalInput")
    qt_d = nc.dram_tensor("qt_out", [S, S], f32, kind="ExternalOutput")

    with tile.TileContext(nc) as tc, ExitStack() as ctx:
        cache_p = ctx.enter_context(tc.tile_pool(name="cache", bufs=1))
        small_p = ctx.enter_context(tc.tile_pool(name="small", bufs=1))
        sig_p = ctx.enter_context(tc.tile_pool(name="sig", bufs=2))
        qt_p = ctx.enter_context(tc.tile_pool(name="qt", bufs=2))
        p_p = ctx.enter_context(tc.tile_pool(name="prod", bufs=3))
        junk_p = ctx.enter_context(tc.tile_pool(name="junk", bufs=2))
        sq_p = ctx.enter_context(tc.tile_pool(name="sq", bufs=2))
        out_p = ctx.enter_context(tc.tile_pool(name="out", bufs=1))

        sconT1_t = small_p.tile([P, S], f32, tag="sc1")
        nc.sync.dma_start(sconT1_t[:], sconT1_d.ap())
        sconT2p_t = small_p.tile([P, 96], f32, tag="sc2")
        nc.sync.dma_start(sconT2p_t[:], sconT2p_d.ap())
        siga_t = sig_p.tile([P, S], f16, tag="siga")
        nc.sync.dma_start(siga_t[:], siga_d.ap())
        sigb_t = sig_p.tile([P, S], f16, tag="sigb")
        nc.sync.dma_start(sigb_t[:], sigb_d.ap())

        cts = []
        for s in range(NSLAB1 + NSLAB2):
            ct = cache_p.tile([P, SLAB], f16, tag=f"c{s}")
            if s < NSLAB1:
                src = c1_d.ap()[:, s * SLAB:(s + 1) * SLAB]
            else:
                src = c2_d.ap()[:, (s - NSLAB1) * SLAB:(s - NSLAB1 + 1) * SLAB]
            nc.sync.dma_start(ct[:], src)
            cts.append(ct)

        for it in range(3):
            qt1 = qt_p.tile([P, S], f32, tag="qt1")
            qt2 = qt_p.tile([P, 96], f32, tag="qt2")
            for s in range(NSLAB1 + NSLAB2):
                is1 = s < NSLAB1
                sig_t = siga_t if is1 else sigb_t
                qt_t = qt1 if is1 else qt2
                base = (s if is1 else s - NSLAB1) * G
                pt = p_p.tile([P, SLAB], f16)
                in0 = cts[s][:].rearrange("p (g k) -> p g k", k=S)
                in1 = sig_t[:].unsqueeze(1).broadcast_to([P, G, S])
                pt3 = pt[:].rearrange("p (g k) -> p g k", k=S)
                nc.vector.tensor_tensor(pt3, in0, in1, mybir.AluOpType.mult)
                d = DVE_SEGS
                if d > 0:
                    nc.vector.tensor_reduce(
                        qt_t[:, base:base + d], pt3[:, 0:d, :],
                        axis=mybir.AxisListType.X, op=mybir.AluOpType.add)
                for g in range(d, G):
                    jt = junk_p.tile([P, S], f16)
                    nc.scalar.activation(
                        jt[:], pt[:, g * S:(g + 1) * S],
                        mybir.ActivationFunctionType.Copy,
                        accum_out=qt_t[:, base + g:base + g + 1])
            nc.vector.tensor_tensor(qt1[:], qt1[:], sconT1_t[:], mybir.AluOpType.add)
            nc.vector.tensor_tensor(qt2[:], qt2[:], sconT2p_t[:], mybir.AluOpType.add)
            if it < 2:
                # xbar transpose needs free dims in multiples of 128 -> pad
                # the sigmoid outputs with filler and transpose 128-blocks.
                sq1 = sq_p.tile([P, 256], f16, tag="sq1")
                sq2 = sq_p.tile([P, 128], f16, tag="sq2")
                Sig = mybir.ActivationFunctionType.Sigmoid
                nc.scalar.activation(sq1[:, 0:S], qt1[:], Sig)
                nc.scalar.activation(sq1[:, S:256], qt1[:, 0:64], Sig)  # filler
                nc.scalar.activation(sq2[:, 0:96], qt2[:], Sig)
                nc.scalar.activation(sq2[:, 96:128], qt2[:, 0:32], Sig)  # filler
                nsa = sig_p.tile([P, S], f16, tag="siga")
                nsb = sig_p.tile([P, S], f16, tag="sigb")
                tmp1 = sq_p.tile([P, 128], f16, tag="tmp1")
                tmp2 = sq_p.tile([P, 128], f16, tag="tmp2")
                nc.sync.dma_start_transpose(nsa[0:128, 0:128], sq1[:, 0:128])
                nc.sync.dma_start_transpose(tmp1[:], sq1[:, 128:256])
                nc.sync.dma_start_transpose(tmp2[:], sq2[:])
                # placement/duplication copies (partition shifts -> DMA)
                nc.scalar.dma_start(nsb[0:64, 0:128], tmp1[0:64, :])
                nc.scalar.dma_start(nsb[64:128, 0:128], tmp1[0:64, :])
                nc.scalar.dma_start(nsa[0:96, 128:192], tmp2[0:96, 0:64])
                nc.scalar.dma_start(nsa[96:128, 128:192], tmp2[0:32, 64:128])
                nc.scalar.dma_start(nsb[0:64, 128:192], tmp2[32:96, 64:128])
                nc.scalar.dma_start(nsb[64:128, 128:192], tmp2[32:96, 64:128])
                siga_t, sigb_t = nsa, nsb
            else:
                o1 = out_p.tile([P, S], f32, tag="o1")
                o2 = out_p.tile([P, 96], f32, tag="o2")
                nc.scalar.activation(o1[:], qt1[:], mybir.ActivationFunctionType.Sigmoid)
                nc.scalar.activation(o2[:], qt2[:], mybir.ActivationFunctionType.Sigmoid)
                nc.sync.dma_start(qt_d.ap()[0:128, :], o1[:])
                nc.sync.dma_start(qt_d.ap()[128:192, 0:96], o2[0:64, :])
                nc.sync.dma_start(qt_d.ap()[128:192, 96:192], o2[64:128, :])
    nc.compile()
    return nc


def _get_program():
    if "nc" not in _CACHE:
        _CACHE["nc"] = _build_program()
    return _CACHE["nc"]


def _prep_core_inputs(s_con_b, sbm16_b):
    """Per-batch input dict. sbm16_b: masked s_bin, fp16, [i, j, k]."""
    A = sbm16_b
    c1 = np.ascontiguousarray(A[:, 0:128, :].transpose(1, 0, 2)).reshape(P, S * S)
    c2a = A[0:96, 128:192, :].transpose(1, 0, 2)     # [64, 96, 192]
    c2b = A[96:192, 128:192, :].transpose(1, 0, 2)   # [64, 96, 192]
    c2 = np.ascontiguousarray(np.concatenate([c2a, c2b], 0)).reshape(P, 96 * S)
    sig1 = (1.0 / (1.0 + np.exp(-s_con_b))).astype(np.float16)   # [a, k] natural
    siga = np.ascontiguousarray(sig1[0:128])
    sigb = np.ascontiguousarray(np.concatenate([sig1[128:192]] * 2, 0))
    sconT = np.ascontiguousarray(s_con_b.T)          # [j, i]
    sconT1 = sconT[0:128].copy()
    sconT2p = np.concatenate([sconT[128:192, 0:96], sconT[128:192, 96:192]], 0).copy()
    return {"c1": c1, "c2": c2, "siga": siga, "sigb": sigb,
            "sconT1": sconT1, "sconT2p": sconT2p}


def kernel(s_con, s_bin, mask):
    from concourse.bass_utils import run_bass_kernel_spmd

    s_con = np.asarray(s_con, dtype=np.float32)
    s_bin = np.asarray(s_bin, dtype=np.float32)
    mask = np.asarray(mask)

    idx = np.arange(S)
    ne = idx[:, None] != idx[None, :]                       # [a, k]
    m2 = ne[:, None, :] & ne[None, :, :]                    # [i, j, k]
    full_mask = mask[:, :, :, None] & m2[None]              # [B, i, j, k]
    sbm16 = (s_bin * full_mask).astype(np.float16)

    nc = _get_program()
    in_maps = [_prep_core_inputs(s_con[b], sbm16[b]) for b in range(B)]
    res = run_bass_kernel_spmd(nc, in_maps, list(range(B)))
    out = np.stack([res.results[b]["qt_out"].T for b in range(B)], 0)
    return np.ascontiguousarray(out.astype(np.float32))
